# revision 26
# baseline (speedup 1.0000x reference)
"""DyGraphTransformer forward on 8 trn2 NeuronCores (Bass/Tile), v2.

Sequence-parallel over N=512 rows (64 per core).  Per layer, the post-LN1
activations y^T (bf16, 32KB) are AllGathered; each core then computes full
K/V locally from the gathered y (replicated weights), so the collective is
small and kicks off right after LN1.

The Graphormer bias is applied as exp(bias) multiplied into exp(scores).
The two tiny embedding tables are renormed+projected+exponentiated on the
host (pure weight preprocessing); the per-(i,j) gather runs on device via
block-diagonal one-hot matmuls (8 j-groups x 8 heads per pass), with the
three gathered tables multiplied together on DVE.

All heavy matmuls run in bf16 (fp32 PSUM accumulate); the residual stream
stays fp32.  ACT stays on the exp_and_others table set (exp/tanh/copies);
LN rsqrt is a DVE bit-trick + Newton.
"""

import sys

sys.path.insert(0, "/opt/trn_rl_repo")

import contextlib

import numpy as np

import concourse.bacc as bacc
import concourse.bass as bass
import concourse.tile as tile
from concourse import mybir
from concourse.bass_utils import run_bass_kernel_spmd

# model dims
N, F, H, NH, L, W = 512, 256, 256, 8, 6, 2
DK = H // NH                 # 32
NC = 8                       # cores
T = N // NC                  # 64 tokens per core
NJT = N // 128               # 4 j-tiles
LN_EPS = 1e-5
SCALE = DK ** -0.5

F32 = mybir.dt.float32
BF16 = mybir.dt.bfloat16
I32 = mybir.dt.int32
AL = mybir.AluOpType
AF = mybir.ActivationFunctionType

GC1 = 0.7978845608028654     # sqrt(2/pi)
GC2 = GC1 * 0.044715
SQ_GC2 = GC2 ** 0.5

NPASS = {"t1a": 2, "t1b": 2, "t2": 8}   # 16-entry subtables

_CACHE = {}


def _gbcast(ap, rep, ncols):
    """AP [G, ncols] -> [G*rep partitions, ncols], each row replicated."""
    g = ap.ap[0][1]
    return bass.AP(tensor=ap.tensor, offset=ap.offset,
                   ap=[[ap.ap[0][0], g], [0, rep], [1, ncols]])


def _bcast_row(dram_ap, p):
    """1-D DRAM AP [Hf] -> broadcast AP [p, Hf]."""
    return bass.AP(tensor=dram_ap.tensor, offset=dram_ap.offset,
                   ap=[[0, p]] + [list(x) for x in dram_ap.ap])


def build(debug=False):
    nc = bacc.Bacc("TRN2", target_bir_lowering=False, debug=False,
                   num_devices=NC)

    # ---------------- DRAM I/O ----------------
    xT_in = nc.dram_tensor("xT", [2, 128, T], F32, kind="ExternalInput")
    xTf_in = nc.dram_tensor("xTf", [2, 128, N], BF16, kind="ExternalInput")
    wfeat_in = nc.dram_tensor("w_feat", [2, 128, H], F32, kind="ExternalInput")
    bfeat_in = nc.dram_tensor("b_feat", [H], F32, kind="ExternalInput")
    ident_in = nc.dram_tensor("identbf", [128, 128], BF16, kind="ExternalInput")
    iota16_in = nc.dram_tensor("iota16", [128], F32, kind="ExternalInput")
    # 10 block-diag lhsT tables: [0:2]=t1 passes, [2:10]=t2 passes
    eblhs_in = nc.dram_tensor("eb_lhs", [128, 10, 64], BF16,
                              kind="ExternalInput")
    # gather indices, bf16 values, [3 tabs, 8 g, 4096 (jj jt i)]
    idx_in = nc.dram_tensor("idx3", [3, 8, 4096], BF16, kind="ExternalInput")

    w_names = ["Wq", "Wk", "Wv", "Wo", "W1", "W2"]
    w_ins = {n: nc.dram_tensor(n, [128, L, 2, H], BF16, kind="ExternalInput")
             for n in w_names}
    b_names = ["bq", "bv"]
    b_ins = {n: nc.dram_tensor(n, [128, L, 2], F32, kind="ExternalInput")
             for n in b_names}
    # row-broadcast biases (per free dim): bo, b1, b2
    brow_in = nc.dram_tensor("brow", [L, 3, H], F32, kind="ExternalInput")

    out_t = nc.dram_tensor("out", [T, H], F32, kind="ExternalOutput")

    CCW = H * T                   # 16384 bf16 = 32KB
    cc_ins = [nc.dram_tensor(f"cc_in{i}", [CCW], BF16) for i in range(L)]
    cc_outs = [nc.dram_tensor(f"cc_out{i}", [NC, CCW], BF16,
                              addr_space="Shared") for i in range(L)]
    ccd_in = nc.dram_tensor("ccd_in", [16], BF16)
    ccd_out = nc.dram_tensor("ccd_out", [NC, 16], BF16, addr_space="Shared")

    with tile.TileContext(nc) as tc:
        ctx = contextlib.ExitStack()
        with ctx:
            const = ctx.enter_context(tc.tile_pool(name="const", bufs=1))
            wpool = ctx.enter_context(tc.tile_pool(name="weights", bufs=1))
            small = ctx.enter_context(tc.tile_pool(name="small", bufs=2))
            psT = ctx.enter_context(tc.tile_pool(name="psT", bufs=2, space="PSUM"))
            psTP = ctx.enter_context(tc.tile_pool(name="psTP", bufs=2, space="PSUM"))

            # ---- dummy collective: warms the ncfw/collective path so the
            # first real AllGathers run at steady-state cost ----
            dummy = const.tile([1, 16], BF16)
            nc.vector.memset(dummy, 0.0)
            nc.sync.dma_start(out=ccd_in.ap().rearrange("(p c) -> p c", p=1),
                              in_=dummy)
            nc.gpsimd.collective_compute(
                "AllGather", AL.bypass, replica_groups=[list(range(NC))],
                ins=[ccd_in[:]], outs=[ccd_out[:, :]])

            # ---- critical-path loads first ----
            ident = const.tile([128, 128], BF16)
            nc.sync.dma_start(out=ident, in_=ident_in[:, :])
            xT = const.tile([128, 2, T], F32)
            nc.sync.dma_start(out=xT, in_=xT_in.ap().rearrange("a p t -> p a t"))
            wfeat = const.tile([128, 2, H], F32)
            nc.sync.dma_start(out=wfeat,
                              in_=wfeat_in.ap().rearrange("a p f -> p a f"))
            xTf = const.tile([128, 2, N], BF16)
            nc.scalar.dma_start(out=xTf,
                                in_=xTf_in.ap().rearrange("a p t -> p a t"))
            wfeat_bf = const.tile([128, 2, H], BF16)
            nc.vector.tensor_copy(out=wfeat_bf.rearrange("p a f -> p (a f)"),
                                  in_=wfeat.rearrange("p a f -> p (a f)"))
            bfeat_bc = const.tile([128, H], F32)
            nc.scalar.dma_start(out=bfeat_bc, in_=_bcast_row(bfeat_in.ap(), 128))
            iota16f = const.tile([128, 1], F32)
            nc.scalar.dma_start(out=iota16f,
                                in_=iota16_in.ap().rearrange("(p o) -> p o", o=1))
            bfeat_r = const.tile([64, H], F32)
            nc.scalar.dma_start(out=bfeat_r, in_=_bcast_row(bfeat_in.ap(), T))
            bsb = {}
            magic = const.tile([128, 1], I32)
            nc.vector.memset(magic, 0x5F3759DF)

            # ---------------- helpers ----------------
            def rsqrt_col(u_ap, p, tagp, iters=1):
                ki = small.tile([128, 1], I32, tag=tagp + "ki")
                nc.vector.tensor_scalar(out=ki[:p], in0=u_ap.bitcast(I32),
                                        scalar1=1, scalar2=None,
                                        op0=AL.logical_shift_right)
                z = small.tile([128, 1], F32, tag=tagp + "z")
                nc.vector.tensor_tensor(out=z[:p].bitcast(I32), in0=magic[:p],
                                        in1=ki[:p], op=AL.subtract)
                t = small.tile([128, 1], F32, tag=tagp + "t")
                for _ in range(iters):
                    nc.vector.tensor_scalar(out=t[:p], in0=z[:p], scalar1=z[:p],
                                            scalar2=u_ap, op0=AL.mult, op1=AL.mult)
                    nc.vector.tensor_scalar(out=t[:p], in0=t[:p], scalar1=-0.5,
                                            scalar2=1.5, op0=AL.mult, op1=AL.add)
                    nc.vector.tensor_tensor(out=z[:p], in0=z[:p], in1=t[:p],
                                            op=AL.mult)
                return z

            def layernorm_stats(h_ap, tagp, p=T):
                stats = small.tile([128, 6], F32, tag=tagp + "st")
                nc.vector.bn_stats(out=stats[:p], in_=h_ap)
                mv = small.tile([128, 2], F32, tag=tagp + "mv")
                nc.vector.bn_aggr(out=mv[:p], in_=stats[:p])
                # eps dropped: var >> 1e-5 here, relative effect < 1e-4
                rstd = rsqrt_col(mv[:p, 1:2], p, tagp)
                return mv, rstd

            # =====================================================
            # Stage A: h0 = x @ Wfeat + b (f32); layer-0 LN1 -> yT -> send
            # =====================================================
            h_sb = const.tile([64, H], F32, tag="resid")
            h_ps = psT.tile([64, H], F32, tag="mm")
            for a in range(2):
                nc.tensor.matmul(h_ps, xT[:, a], wfeat[:, a],
                                 start=(a == 0), stop=(a == 1))
            nc.vector.tensor_tensor(out=h_sb, in0=h_ps, in1=bfeat_r,
                                    op=AL.add)

            yT_sb = const.tile([128, 2, T], BF16, tag="yT")
            y0 = const.tile([64, H], BF16, tag="y0")

            def ln1_to_yT(l, send=True):
                mv, rstd = layernorm_stats(h_sb, "ln1")
                nc.vector.tensor_scalar(out=y0, in0=h_sb,
                                        scalar1=mv[:T, 0:1], scalar2=rstd[:T],
                                        op0=AL.subtract, op1=AL.mult)
                for a in range(2):
                    tp = psTP.tile([128, T], BF16, tag="tp")
                    nc.tensor.transpose(tp, y0[:, 128 * a:128 * (a + 1)],
                                        ident[:T, :T])
                    # ln1 affine folded into Wq/Wk/Wv (host)
                    nc.vector.tensor_copy(out=yT_sb[:, a], in_=tp)
                if send:
                    nc.sync.dma_start(
                        out=cc_ins[l].ap().rearrange("(p c) -> p c", p=128),
                        in_=yT_sb.rearrange("p a t -> p (a t)"))
                    nc.gpsimd.collective_compute(
                        "AllGather", AL.bypass,
                        replica_groups=[list(range(NC))],
                        ins=[cc_ins[l][:]], outs=[cc_outs[l][:, :]])

            ln1_to_yT(0, send=False)

            # layer 0 needs no collective: every core computes the full
            # y0 from the (replicated) full x input
            yg0 = const.tile([128, 2, N], BF16, tag="yg0")
            for pt in range(4):
                hp_ps = psT.tile([128, H], F32, tag="mm")
                for a in range(2):
                    nc.tensor.matmul(hp_ps,
                                     xTf[:, a, 128 * pt:128 * (pt + 1)],
                                     wfeat_bf[:, a],
                                     start=(a == 0), stop=(a == 1))
                h0f = small.tile([128, H], F32, tag="h0f")
                nc.vector.tensor_tensor(out=h0f, in0=hp_ps, in1=bfeat_bc,
                                        op=AL.add)
                mvf, rstdf = layernorm_stats(h0f, "ln0f", p=128)
                y0f = small.tile([128, H], BF16, tag="y0f")
                nc.vector.tensor_scalar(out=y0f, in0=h0f,
                                        scalar1=mvf[:, 0:1], scalar2=rstdf,
                                        op0=AL.subtract, op1=AL.mult)
                for a in range(2):
                    tpf = psTP.tile([128, 128], BF16, tag="tp")
                    nc.tensor.transpose(tpf, y0f[:, 128 * a:128 * (a + 1)],
                                        ident)
                    nc.vector.tensor_copy(out=yg0[:, a, 128 * pt:128 * (pt + 1)],
                                          in_=tpf)

            # =====================================================
            # Stage B: bulk loads + bias gather (overlaps barrier + AG0)
            # =====================================================
            bctx = contextlib.ExitStack()
            bb = bctx.enter_context(tc.tile_pool(name="biasbuild", bufs=1))
            ohp = bctx.enter_context(tc.tile_pool(name="ohp", bufs=2))
            psG = bctx.enter_context(tc.tile_pool(name="psG", bufs=2, space="PSUM"))

            eblhs = const.tile([128, 10, 64], BF16)
            nc.sync.dma_start(out=eblhs, in_=eblhs_in[:, :, :])
            idx_t = {}
            for k, tab in enumerate(["t1a", "t1b", "t2"]):
                it = bb.tile([128, 4096], BF16, tag="idx_" + tab,
                             name="idx_" + tab)
                eng = nc.sync if tab != "t1b" else nc.scalar
                eng.dma_start(out=it, in_=_gbcast(idx_in.ap()[k], 16, 4096))
                idx_t[tab] = it
            # remaining weights/biases, balanced across the two rings
            wsb = {}
            for i, n in enumerate(["Wq", "Wk", "Wv", "Wo", "W1", "W2"]):
                tl = wpool.tile([128, L, 2, H], BF16, tag="w_" + n,
                                name="w_" + n)
                eng = nc.scalar if i % 2 == 0 else nc.sync
                eng.dma_start(out=tl, in_=w_ins[n][:, :, :, :])
                wsb[n] = tl
            for n in ["bq", "bv"]:
                tl = wpool.tile([128, L, 2], F32, tag="b_" + n, name="b_" + n)
                nc.scalar.dma_start(out=tl, in_=b_ins[n][:, :, :])
                bsb[n] = tl
            brow = wpool.tile([64, L, 3, H], F32, tag="brow")
            nc.sync.dma_start(out=brow, in_=_bcast_row(
                brow_in.ap().rearrange("l k f -> (l k f)"), T).rearrange(
                    "p (l k f) -> p l k f", l=L, k=3))

            # gather: all 12 one-hot passes accumulate into one PSUM tile,
            # a single eviction yields the summed raw bias [64=(8h+g), 4096]
            bias_hm = bb.tile([64, 4096], BF16, tag="bias_hm")
            passes = [("t1a", 0, 0), ("t1a", 0, 1),
                      ("t1b", 0, 0), ("t1b", 0, 1)] + \
                     [("t2", 2, q) for q in range(8)]
            for ch in range(4):              # 1024-col chunks
                ps = psG.tile([64, 1024], F32, tag="g")
                for pi, (tab, lhs0, q) in enumerate(passes):
                    oh = ohp.tile([128, 1024], BF16, tag="oh")
                    nc.vector.tensor_scalar(
                        out=oh, in0=idx_t[tab][:, 1024 * ch:1024 * (ch + 1)],
                        scalar1=float(16 * q), scalar2=iota16f,
                        op0=AL.subtract, op1=AL.is_equal)
                    for hf in range(2):
                        nc.tensor.matmul(
                            ps[:, 512 * hf:512 * (hf + 1)],
                            eblhs[:, lhs0 + q],
                            oh[:, 512 * hf:512 * (hf + 1)],
                            start=(pi == 0), stop=(pi == len(passes) - 1))
                nc.scalar.activation(bias_hm[:, 1024 * ch:1024 * (ch + 1)],
                                     ps, AF.Copy)

            # reorient to eb [128 j, (m, jt, h', i)] (raw bias, bf16)
            eb = const.tile([128, 2, NJT, 4, T], BF16, tag="eb")
            for h in range(NH):
                m, hp = h // 4, h % 4
                eng = nc.sync if h % 2 == 0 else nc.scalar
                eng.dma_start(
                    out=eb[:, m, :, hp, :],
                    in_=bias_hm[8 * h:8 * h + 8].rearrange(
                        "g (jj r) -> g jj r", jj=16))

            bctx.close()

            # =====================================================
            # Stage C: layers
            # =====================================================
            lctx = contextlib.ExitStack()
            work = lctx.enter_context(tc.tile_pool(name="work", bufs=2))
            psS = lctx.enter_context(tc.tile_pool(name="psS", bufs=2, space="PSUM"))

            # persistent attention tiles
            qbdA = const.tile([128, 4 * T], BF16, tag="qbdA")
            nc.vector.memset(qbdA, 0.0)
            qbdB = const.tile([128, 4 * T], BF16, tag="qbdB")
            nc.vector.memset(qbdB, 0.0)
            qbd = [qbdA, qbdB]
            vtA = const.tile([128, 2, NH, 33], BF16, tag="vtA")
            nc.vector.memset(vtA.rearrange("p j h d -> p (j h d)"), 1.0)
            vtB = const.tile([128, 2, NH, 33], BF16, tag="vtB")
            nc.vector.memset(vtB.rearrange("p j h d -> p (j h d)"), 1.0)
            vt = [vtA, vtB]   # vt[jt % 2][:, jt // 2]

            for l in range(L):
                if l > 0:
                    ln1_to_yT(l)

                # ---- q block-diag (overlaps AG) ----
                q_ps = psT.tile([128, 2, T], F32, tag="mm")
                for m2 in range(2):
                    for a in range(2):
                        nc.tensor.matmul(
                            q_ps[:, m2],
                            wsb["Wq"][:, l, a, 128 * m2:128 * (m2 + 1)],
                            yT_sb[:, a], start=(a == 0), stop=(a == 1))
                for h in range(NH):
                    m2, hp = h // 4, h % 4
                    dst = qbd[m2][32 * hp:32 * hp + 32, T * hp:T * hp + T]
                    sp = q_ps[32 * hp:32 * hp + 32, m2]
                    bq_ap = bsb["bq"][32 * hp:32 * hp + 32, l, m2:m2 + 1]
                    if m2 == 0:
                        nc.vector.tensor_scalar(out=dst, in0=sp, scalar1=bq_ap,
                                                scalar2=None, op0=AL.add)
                    else:
                        nc.scalar.activation(dst, sp, AF.Identity, bias=bq_ap,
                                             scale=1.0)
                # constant residual biases: independent of attention output,
                # applied here while DVE idles in the collective window
                nc.vector.tensor_tensor(out=h_sb, in0=h_sb,
                                        in1=brow[:, l, 0], op=AL.add)
                if l > 0:
                    # keep the PE activity window hot through the collective
                    # so the post-AG matmul burst runs at 2.4 GHz
                    for w in range(22):
                        jp = psT.tile([128, H], F32, tag="mm", name="junk")
                        nc.tensor.matmul(jp, ident, wsb["Wo"][:, l, 0],
                                         start=True, stop=True)

                # ---- bias preload into scores PSUM (PE work in AG window)
                probs = work.tile([128, 2, NJT, 4, T], BF16, tag="probs")
                s_pss = []
                for m2 in range(2):
                    s_ps = psS.tile([128, NJT, 4, T], F32, tag="sc",
                                    name="s_ps")
                    s_pss.append(s_ps)
                    for jt in range(NJT):
                        # identity lhsT: I^T @ eb = eb
                        nc.tensor.matmul(
                            s_ps[:, jt], ident,
                            eb[:, m2, jt].rearrange("p h t -> p (h t)"),
                            start=True, stop=False)

                # ---- AG lands: read back gathered yT (split by core-half
                # so K/V matmuls start on the first half early) ----
                if l == 0:
                    ygT = yg0
                else:
                    ygT = work.tile([128, 2, N], BF16, tag="ygT")
                    for ch in range(2):
                        for a in range(2):
                            eng = nc.sync if a == 0 else nc.scalar
                            eng.dma_start(
                                out=ygT[:, a, 256 * ch:256 * (ch + 1)]
                                .rearrange("p (c t) -> p c t", c=4),
                                in_=cc_outs[l].ap().rearrange(
                                    "c (p a t) -> p a c t", p=128,
                                    a=2)[:, a, 4 * ch:4 * (ch + 1)])

                # ---- full K^T (k-bias dropped: softmax-invariant) ----
                kTt = [work.tile([128, N], BF16, tag="kTA", name="kTA"),
                       work.tile([128, N], BF16, tag="kTB", name="kTB")]
                for m2 in range(2):
                    k_ps = psT.tile([128, N], F32, tag="mm")
                    for ch in range(2):
                        for a in range(2):
                            nc.tensor.matmul(
                                k_ps[:, 256 * ch:256 * (ch + 1)],
                                wsb["Wk"][:, l, a, 128 * m2:128 * (m2 + 1)],
                                ygT[:, a, 256 * ch:256 * (ch + 1)],
                                start=(a == 0), stop=(a == 1))
                    if m2 == 0:
                        nc.vector.tensor_copy(out=kTt[m2], in_=k_ps)
                    else:
                        nc.scalar.activation(kTt[m2], k_ps, AF.Copy)

                # ---- full V (token-major per j-tile), ones col persists ----
                for jt in range(NJT):
                    v_ps = psT.tile([128, H], F32, tag="mm")
                    for a in range(2):
                        nc.tensor.matmul(
                            v_ps, ygT[:, a, 128 * jt:128 * (jt + 1)],
                            wsb["Wv"][:, l, a], start=(a == 0), stop=(a == 1))
                    if jt % 2 == 0:
                        nc.vector.tensor_copy(
                            out=vt[0][:, jt // 2, :, 0:32],
                            in_=v_ps.rearrange("p (h d) -> p h d", h=NH))
                    else:
                        nc.scalar.activation(
                            vt[1][:, jt // 2, :, 0:32],
                            v_ps.rearrange("p (h d) -> p h d", h=NH), AF.Copy)

                # ---- scores + exp + bias-mult ----
                for m2 in range(2):
                    s_ps = s_pss[m2]
                    for jt in range(NJT):
                        nc.tensor.matmul(
                            s_ps[:, jt],
                            kTt[m2][:, 128 * jt:128 * (jt + 1)],
                            qbd[m2], start=False, stop=True)
                    nc.scalar.activation(
                        probs[:, m2].rearrange("p j h t -> p (j h t)"),
                        s_ps.rearrange("p j h t -> p (j h t)"), AF.Exp)

                # ---- A@V with ones-col row sums ----
                o_ps = psT.tile([64, NH, 33], F32, tag="mm")
                for h in range(NH):
                    m2, hp = h // 4, h % 4
                    for jt in range(NJT):
                        nc.tensor.matmul(o_ps[:, h], probs[:, m2, jt, hp],
                                         vt[jt % 2][:, jt // 2, h],
                                         start=(jt == 0), stop=(jt == NJT - 1))
                rec = small.tile([64, NH], F32, tag="rec")
                nc.vector.reciprocal(out=rec, in_=o_ps[:, :, 32])
                o_sb = work.tile([64, H], BF16, tag="o_sb")
                rb = rec[:, 0:NH]
                rec_b = bass.AP(tensor=rb.tensor, offset=rb.offset,
                                ap=[list(rb.ap[0]), list(rb.ap[1]), [0, 32]])
                nc.vector.tensor_tensor(
                    out=o_sb.rearrange("p (h d) -> p h d", h=NH),
                    in0=o_ps[:, :, 0:32], in1=rec_b, op=AL.mult)

                # ---- h += (o + bv-fold) @ Wo + bo ----
                oT = work.tile([128, 2, T], BF16, tag="oT")
                for a in range(2):
                    tp = psTP.tile([128, T], BF16, tag="tp")
                    nc.tensor.transpose(tp, o_sb[:, 128 * a:128 * (a + 1)],
                                        ident[:T, :T])
                    nc.scalar.activation(oT[:, a], tp, AF.Identity,
                                         bias=bsb["bv"][:, l, a:a + 1],
                                         scale=1.0)
                at_ps = psT.tile([64, H], F32, tag="mm")
                for a in range(2):
                    nc.tensor.matmul(at_ps, oT[:, a], wsb["Wo"][:, l, a],
                                     start=(a == 0), stop=(a == 1))
                nc.vector.tensor_tensor(out=h_sb, in0=h_sb, in1=at_ps,
                                        op=AL.add)

                # ---- LN2 + FFN ----
                mv2, rstd2 = layernorm_stats(h_sb, "ln2")
                y2 = work.tile([64, H], BF16, tag="y2")
                nc.vector.tensor_scalar(out=y2, in0=h_sb,
                                        scalar1=mv2[:T, 0:1], scalar2=rstd2[:T],
                                        op0=AL.subtract, op1=AL.mult)
                # W2 bias (post-FFN residual term); h free until the
                # attn-resid below, y2 already extracted above
                nc.vector.tensor_tensor(out=h_sb, in0=h_sb,
                                        in1=brow[:, l, 2], op=AL.add)
                y2T = work.tile([128, 2, T], BF16, tag="y2T")
                for a in range(2):
                    tp = psTP.tile([128, T], BF16, tag="tp")
                    nc.tensor.transpose(tp, y2[:, 128 * a:128 * (a + 1)],
                                        ident[:T, :T])
                    # ln2 affine folded into W1 (host)
                    nc.vector.tensor_copy(out=y2T[:, a], in_=tp)
                z_ps = psT.tile([64, H], F32, tag="mm")
                for a in range(2):
                    nc.tensor.matmul(z_ps, y2T[:, a], wsb["W1"][:, l, a],
                                     start=(a == 0), stop=(a == 1))
                z_sb = work.tile([64, H], BF16, tag="z")
                nc.vector.tensor_tensor(out=z_sb, in0=z_ps, in1=brow[:, l, 1],
                                        op=AL.add)
                # tanh-gelu: gg = z * (0.5 + 0.5*tanh(z*(GC1 + GC2 z^2)))
                z2 = work.tile([64, H], BF16, tag="z2")
                nc.vector.tensor_tensor(out=z2, in0=z_sb, in1=z_sb, op=AL.mult)
                zg = work.tile([64, H], BF16, tag="zg")
                nc.vector.tensor_scalar(out=zg, in0=z2, scalar1=GC2,
                                        scalar2=GC1, op0=AL.mult, op1=AL.add)
                gu = work.tile([64, H], BF16, tag="gu")
                nc.vector.tensor_tensor(out=gu, in0=zg, in1=z_sb, op=AL.mult)
                gt = work.tile([64, H], BF16, tag="gt")
                nc.scalar.activation(gt, gu, AF.Tanh)
                gh = work.tile([64, H], BF16, tag="gh")
                nc.vector.tensor_scalar(out=gh, in0=gt, scalar1=1.0,
                                        scalar2=None, op0=AL.add)
                gg = work.tile([64, H], BF16, tag="gg")
                nc.vector.tensor_tensor(out=gg, in0=gh, in1=z_sb, op=AL.mult)
                # (the 0.5 factor is folded into W2 on the host)
                gT = work.tile([128, 2, T], BF16, tag="gT")
                for a in range(2):
                    tp = psTP.tile([128, T], BF16, tag="tp")
                    nc.tensor.transpose(tp, gg[:, 128 * a:128 * (a + 1)],
                                        ident[:T, :T])
                    nc.vector.tensor_copy(out=gT[:, a], in_=tp)
                f_ps = psT.tile([64, H], F32, tag="mm")
                for a in range(2):
                    nc.tensor.matmul(f_ps, gT[:, a], wsb["W2"][:, l, a],
                                     start=(a == 0), stop=(a == 1))
                nc.vector.tensor_tensor(out=h_sb, in0=h_sb, in1=f_ps,
                                        op=AL.add)

            nc.sync.dma_start(out=out_t[:, :], in_=h_sb)
            lctx.close()

    nc.compile()
    return nc


# ---------------- host marshalling ----------------

def _prep_inputs(inputs):
    import ml_dtypes

    BF = ml_dtypes.bfloat16

    def f32(a):
        return np.ascontiguousarray(np.asarray(a, np.float32))

    def bf16(a):
        return np.ascontiguousarray(np.asarray(a).astype(BF))

    x = f32(inputs["x"])
    ee = np.asarray(inputs["edge_encodes"]).astype(np.int64)
    ede = np.asarray(inputs["edge_dist_encodes"]).astype(np.int64)[:, :, 0]

    # --- weight preprocessing: renorm + project + exp the bias tables ---
    def renorm(t):
        t = np.asarray(t, np.float64)
        n = np.linalg.norm(t, axis=-1, keepdims=True)
        return t * np.where(n > 1.0, 1.0 / (n + 1e-7), 1.0)

    p1 = renorm(inputs["edge_emb"]) @ np.asarray(inputs["W_ee"], np.float64)
    p2 = renorm(inputs["edge_dist_emb"]) @ np.asarray(inputs["W_ed"], np.float64)
    t1v = 0.5 * p1                                             # [32, 8]
    t2v = p2 + np.asarray(inputs["b_ee"], np.float64) \
        + np.asarray(inputs["b_ed"], np.float64)               # [128, 8]

    # block-diag lhsT tiles [10, 128, 64]: [q][16g+e', 8h+g]
    eb_lhs = np.zeros((10, 8, 16, 8, 8), np.float64)
    for q in range(2):
        for g in range(8):
            eb_lhs[q, g, :, :, g] = t1v[16 * q:16 * q + 16]
    for q in range(8):
        for g in range(8):
            eb_lhs[2 + q, g, :, :, g] = t2v[16 * q:16 * q + 16]
    # -> [128, 10, 64] partition-major for a contiguous load
    eb_lhs = np.ascontiguousarray(
        eb_lhs.reshape(10, 128, 64).transpose(1, 0, 2))

    def wprep(w):
        # [L, H, H] -> [128 p, L, 2 a, H] partition-major contiguous
        return np.ascontiguousarray(
            np.asarray(w).reshape(L, 2, 128, H).transpose(2, 0, 1, 3))

    def bprep(b):
        # [L, H] -> [128 p, L, 2 a]
        return np.ascontiguousarray(
            np.asarray(b, np.float32).reshape(L, 2, 128).transpose(2, 0, 1))

    # fold LN affines into the consuming projections:
    #   q/k/v consume ln1(y)*s+b; ffn-in consumes ln2(.)*s+b
    s1 = np.asarray(inputs["ln1_s"], np.float64)[:, :, None]   # [L, H, 1]
    b1n = np.asarray(inputs["ln1_b"], np.float64)              # [L, H]
    s2 = np.asarray(inputs["ln2_s"], np.float64)[:, :, None]
    b2n = np.asarray(inputs["ln2_b"], np.float64)
    Wq = np.asarray(inputs["Wq"], np.float64)
    Wk = np.asarray(inputs["Wk"], np.float64)
    Wv = np.asarray(inputs["Wv"], np.float64)
    W1 = np.asarray(inputs["W1"], np.float64)
    bq_f = (np.einsum("lh,lho->lo", b1n, Wq)
            + np.asarray(inputs["bq"], np.float64)) * SCALE
    bv_f = (np.einsum("lh,lho->lo", b1n, Wv)
            + np.asarray(inputs["bv"], np.float64))
    b1_f = (np.einsum("lh,lho->lo", b2n, W1)
            + np.asarray(inputs["b1"], np.float64))
    # (k-bias is softmax-invariant and dropped)
    shared = {
        "w_feat": f32(inputs["W_feat"]).reshape(2, 128, H),
        "b_feat": f32(inputs["b_feat"]),
        "identbf": np.eye(128, dtype=np.float32).astype(BF),
        "xTf": np.ascontiguousarray(x.T).reshape(2, 128, N).astype(BF),
        "iota16": (np.arange(128) % 16).astype(np.float32),
        "eb_lhs": eb_lhs.astype(BF),
        "bq": bprep(bq_f),
        "bv": bprep(bv_f),
        "Wq": wprep(s1 * Wq * SCALE).astype(BF),
        "Wk": wprep(s1 * Wk).astype(BF),
        "Wv": wprep(s1 * Wv).astype(BF),
        "W1": wprep(s2 * W1).astype(BF),
        "Wo": bf16(wprep(inputs["Wo"])),
        # 0.5 of tanh-gelu folded into W2
        "W2": wprep(np.asarray(inputs["W2"], np.float64) * 0.5).astype(BF),
    }
    shared["brow"] = np.ascontiguousarray(np.stack(
        [f32(inputs["bo"]), f32(b1_f.astype(np.float64)),
         f32(inputs["b2"])], axis=1))

    in_maps = []
    for c in range(NC):
        rows = slice(T * c, T * (c + 1))
        m = dict(shared)
        m["xT"] = np.ascontiguousarray(x[rows].T).reshape(2, 128, T)
        # idx layout [tab, 8 g, (16 jj, 4 jt, 64 i)], j = jt*128 + g*16 + jj
        idx3 = np.empty((3, 8, 16, NJT, T), np.float32)
        for k, arr in enumerate([ee[rows, :, 0], ee[rows, :, 1], ede[rows]]):
            # arr [64 i, 512 j] -> [jt, g, jj, i] -> [g, jj, jt, i]
            a4 = arr.T.reshape(NJT, 8, 16, T).transpose(1, 2, 0, 3)
            idx3[k] = a4
        m["idx3"] = np.ascontiguousarray(idx3.reshape(3, 8, 4096)).astype(BF)
        in_maps.append(m)
    return in_maps


def kernel(**inputs):
    debug = inputs.pop("_debug", False)
    trace = inputs.pop("_trace", False)
    tmpdir = inputs.pop("_tmpdir", None)
    key = ("k", debug)
    if key not in _CACHE:
        _CACHE[key] = build(debug=debug)
    nc = _CACHE[key]
    in_maps = _prep_inputs(inputs)
    res = run_bass_kernel_spmd(nc, in_maps, list(range(NC)), trace=trace,
                               tmpdir=tmpdir)
    kernel._last = res
    out = np.concatenate([res.results[c]["out"] for c in range(NC)], axis=0)
    return out
